# revision 31
# baseline (speedup 1.0000x reference)
"""Trainium2 Bass kernel for ConditionalEdgeDenoiser (GNN edge MLP denoiser).

Reference computation (per batch b, nodes i,j):
    h = concat([edge_t[b,i,j,:],            # 4   (EC)
                node_ctx[b,i,:],            # 80  (src = x_cond||code_cond)
                node_ctx[b,j,:],            # 80  (dst)
                time_emb[b,:]])             # 128 (TDIM)
    h1 = silu(h @ W1 + b1); h2 = silu(h1 @ W2 + b2); out = (h2 @ W3 + b3) * mask

Strategy (8 cores, data-parallel over (B x row-halves) = 8 shards of 128 rows):
  * Same augmented layer-1 matmul as before (stationary stacks
    [W1_edge; W1_dst; srcbias rows], srcbias = nctx@W1_src + temb@W1_time + b1
    precomputed on host), fp32r everywhere, PSUM accumulates fp32.
  * ScalarE (silu) is the bottleneck engine: its cost is free-size x 0.83ns
    + a fixed ~185ns bubble per instruction.  So layer-1's silu is ONE
    [128, 2048] op over a single 8KB-per-partition PSUM tile (both hid
    halves side by side); layer-2's silu stays as two [128, 1024] ops
    because the activation bias (b2) is per-partition and differs between
    hid halves.  3 activation ops/tile instead of 4.
  * PSUM budget (16KB/partition) is exactly p1[128,2048] + p2a/p2b[128,1024].
    The tiny L3 output (p3 [4,1024]) aliases p2a's buffer: L3(k-2) runs
    after silu2a(k-1) has drained p2a, and the Pool engine copies p3+b3 out
    to SBUF in two [4,512] halves so the buffer is free again before
    L2h0(k) needs it.  Emission order per iteration interleaves three
    pipeline stages so ScalarE never waits:
      L2h0(k-1), silu1(k), L2h1(k-1), silu2a(k-1), L1(k+1), silu2b(k-1),
      L3(k-2), pool-copies(k-2), out-dma(k-2).
  * node_mask handling moved off-device: the reference multiplies the output
    by mask_i*mask_j; kernel() applies it on the host only when the mask is
    not all-ones (it is all-ones in setup_inputs).  DVE does no work at all.
"""

import os
import sys

sys.path.insert(0, "/opt/trn_rl_repo")
os.environ.setdefault("MYCRO_LOCAL_CACHE", "1")

import numpy as np

import concourse.bass as bass  # noqa: E402
import concourse.mybir as mybir  # noqa: E402
import concourse.tile as tile  # noqa: E402
from concourse import bacc  # noqa: E402
from concourse.bass_utils import run_bass_kernel_spmd  # noqa: E402

B, N, EC, FEAT, CODE, HID, TDIM = 4, 256, 4, 64, 16, 256, 128
NCTX = FEAT + CODE  # 80
NCORES = 8
RPT = 4                      # grid rows per tile
E = RPT * N                  # 1024 edge columns per tile
CH = 512                     # matmul moving-dim chunk (fp32 PSUM bank limit)
NCH = E // CH                # chunks per tile
ROWS = N // 2                # 128 grid rows per core
NT = ROWS // RPT             # 32 tiles per core
KAUG = EC + NCTX + RPT       # 88 = augmented contraction dim for layer 1

F32 = mybir.dt.float32
F32R = mybir.dt.float32r
AF = mybir.ActivationFunctionType
ALU = mybir.AluOpType

DS = 512                     # silu2b tail columns offloaded to a DVE chain
SCH_A = float(2 ** 23 / np.log(2.0))
SCH_B = float(127 * 2 ** 23 - 486411)
I32 = mybir.dt.int32

_CACHE = {}


def _build():
    nc = bacc.Bacc("TRN2", debug=False, num_devices=NCORES)

    # ---- DRAM I/O (per core) ----
    edge_d = nc.dram_tensor("edge", [NT, EC, E], F32R, kind="ExternalInput")
    srcb_d = nc.dram_tensor("srcb", [ROWS, HID], F32R, kind="ExternalInput")
    w1ed_d = nc.dram_tensor("w1ed", [EC + NCTX, HID], F32R, kind="ExternalInput")
    b2c_d = nc.dram_tensor("b2c", [128, 4], F32, kind="ExternalInput")
    b3r_d = nc.dram_tensor("b3r", [128, 8 * EC], F32, kind="ExternalInput")
    w2_d = nc.dram_tensor("w2", [HID, HID], F32R, kind="ExternalInput")
    w3_d = nc.dram_tensor("w3", [HID, EC], F32R, kind="ExternalInput")
    rhsstat_d = nc.dram_tensor("rhsstat", [NCTX + RPT, E], F32R, kind="ExternalInput")
    out_d = nc.dram_tensor("out", [NT, 128, 8 * EC], F32, kind="ExternalOutput")

    with tile.TileContext(nc) as tc:
        with tc.tile_pool(name="const", bufs=1) as cp, \
             tc.tile_pool(name="h", bufs=4) as hp, \
             tc.tile_pool(name="o", bufs=6) as op, \
             tc.tile_pool(name="sc", bufs=3) as sp, \
             tc.tile_pool(name="ps", bufs=1, space="PSUM") as pp:

            # ---------- augmented layer-1 operands (ping-pong pairs) ----------
            # lh[q]: [KAUG, 256] stationary tile, halves at cols 0:128 / 128:256.
            # rhs_t[q]: [KAUG, E] moving tile.  These feed tile 0, so their
            # loads come first and are merged into one DMA per buffer.
            NB = 3   # lh/rhs operand buffers: 3 so prefetch never hits WAR
            lh = [None] * NB
            rhs_t = [None] * NB
            for q in range(NB):
                lh[q] = cp.tile([KAUG, HID], F32R, tag=f"lh{q}", name=f"lh{q}")
                rhs_t[q] = cp.tile([KAUG, E], F32R, tag=f"rhs{q}", name=f"rhs{q}")
            # PE p-state warmup: dummy matmuls on a memset tile keep the
            # PE busy during the initial DMA wait, so the first real L1 runs
            # at full clock (213ns/mm) instead of cold (427ns/mm).
            wsrc = cp.tile([KAUG, CH], F32, tag="warm")
            nc.vector.memset(wsrc, 0.0)
            pwarm = pp.tile([128, CH], F32, name="pwarm", tag="p1")
            for _ in range(4):
                nc.tensor.matmul(pwarm, lhsT=wsrc[:, 0:128], rhs=wsrc,
                                 start=True, stop=True)

            # tile-0's critical path, longest DMA first (HWDGE launches are
            # ~650ns apart, so launch order bounds when L1(0) can start).
            nc.sync.dma_start(out=rhs_t[0][EC:KAUG, :], in_=rhsstat_d[:])
            nc.gpsimd.dma_start(out=lh[0][EC + NCTX:KAUG, :], in_=srcb_d[0:RPT])
            nc.sync.dma_start(out=lh[0][0:EC + NCTX, :], in_=w1ed_d[:])
            nc.sync.dma_start(out=rhs_t[0][0:EC, :], in_=edge_d[0])
            # tile-1 operands next so L1(1) isn't stuck behind the q=2 statics
            nc.sync.dma_start(out=rhs_t[1][EC:KAUG, :], in_=rhsstat_d[:])
            nc.sync.dma_start(out=rhs_t[1][0:EC, :], in_=edge_d[1])
            nc.gpsimd.dma_start(out=lh[1][EC + NCTX:KAUG, :],
                                in_=srcb_d[RPT:2 * RPT])
            nc.sync.dma_start(out=lh[1][0:EC + NCTX, :], in_=w1ed_d[:])
            nc.sync.dma_start(out=lh[2][0:EC + NCTX, :], in_=w1ed_d[:])
            nc.sync.dma_start(out=rhs_t[2][EC:KAUG, :], in_=rhsstat_d[:])

            w2k0 = w2k1 = w30 = w31 = b2c = b3r = None
            h1s, h2s = {}, {}
            p1s, p2as, p2bs, p2cs, p3s, ots = {}, {}, {}, {}, {}, {}

            def load_consts():
                nonlocal w2k0, w2k1, w30, w31, b2c, b3r
                w2k0 = cp.tile([128, HID], F32R, tag="w2k0")
                nc.gpsimd.dma_start(out=w2k0, in_=w2_d[0:128])
                w2k1 = cp.tile([128, HID], F32R, tag="w2k1")
                nc.gpsimd.dma_start(out=w2k1, in_=w2_d[128:256])
                w30 = cp.tile([128, EC], F32R, tag="w30")
                nc.sync.dma_start(out=w30, in_=w3_d[0:128])
                w31 = cp.tile([128, EC], F32R, tag="w31")
                nc.sync.dma_start(out=w31, in_=w3_d[128:256])
                b2c = cp.tile([128, 4], F32, tag="b2c")
                nc.gpsimd.dma_start(out=b2c, in_=b2c_d[:])
                b3r = cp.tile([128, 8 * EC], F32, tag="b3r")
                nc.sync.dma_start(out=b3r, in_=b3r_d[:])

            def emit_L1(j):
                # layer-1 matmuls for tile j into a single [128, 2*E] PSUM
                # tile.  Tile 0 instead writes its halves into the then-unused
                # p2a/p2b buffers: p1 stays free for L1(1) during the fill,
                # and silu1(0)a only depends on the h0 matmuls.
                lht, rhs = lh[j % NB], rhs_t[j % NB]
                if j == 0:
                    pz = [pp.tile([128, E], F32, name="z0a", tag="p2a"),
                          pp.tile([128, CH], F32, name="z0b", tag="p2b"),
                          pp.tile([128, CH], F32, name="z0c", tag="p2c")]
                else:
                    p1 = pp.tile([128, 2 * E], F32, name=f"p1_{j}", tag="p1")
                for h in range(2):
                    for c in range(NCH):
                        if j == 0:
                            dst = (pz[0][:, c * CH:(c + 1) * CH] if h == 0
                                   else pz[1 + c])
                        else:
                            dst = p1[:, h * E + c * CH:h * E + (c + 1) * CH]
                        nc.tensor.matmul(
                            dst,
                            lhsT=lht[:, h * 128:(h + 1) * 128],
                            rhs=rhs[:, c * CH:(c + 1) * CH],
                            start=True, stop=True)
                p1s[j] = pz if j == 0 else p1

            # ---------- main loop: 3-stage software pipeline ----------
            # Iteration k emits: L2h0(k-1), silu1(k), L2h1(k-1), silu2a(k-1),
            # L1(k+1), silu2b(k-1), L3(k-2), copies(k-2), dma(k-2).
            emit_L1(0)
            for k in range(NT + 2):
                # input loads for tile k+1 (double-buffered operands)
                if 1 <= k and k + 1 < NT:
                    rhs = rhs_t[(k + 1) % NB]
                    nc.sync.dma_start(out=rhs[0:EC, :], in_=edge_d[k + 1])
                    nc.gpsimd.dma_start(
                        out=lh[(k + 1) % NB][EC + NCTX:KAUG, :],
                        in_=srcb_d[RPT * (k + 1):RPT * (k + 2)])
                if k == 1:
                    load_consts()

                # ---- L2 half 1, p2c chunk, for tile k-1 (FIRST on PE):
                # it feeds the DVE silu chain, whose finish time gates the
                # next tile's L3-tail/copy/L2h0 relay ----
                if 1 <= k <= NT and k - 1 < NT - 1:
                    j = k - 1
                    h1 = h1s[j]
                    p2c = pp.tile([128, CH], F32, name=f"p2c_{j}", tag="p2c")
                    p2cs[j] = p2c
                    nc.tensor.matmul(p2c, lhsT=w2k0[:, 128:256],
                                     rhs=h1[:, CH:2 * CH],
                                     start=True, stop=False)
                    nc.tensor.matmul(p2c, lhsT=w2k1[:, 128:256],
                                     rhs=h1[:, E + CH:E + 2 * CH],
                                     start=False, stop=True)

                # ---- L2 half 0 for tile k-1 ----
                # (last tile: both z2 halves go into the then-free p1 buffer,
                # so L2(NT-1) skips the p2a/p2b copy-chain WAR gates)
                if 1 <= k <= NT:
                    j = k - 1
                    h1 = h1s[j]
                    if j == NT - 1:
                        p2a = pp.tile([128, E], F32, name="p2fa", tag="p1")
                    else:
                        p2a = pp.tile([128, E], F32, name=f"p2a_{j}", tag="p2a")
                    for c in range(NCH):
                        dst = p2a[:, c * CH:(c + 1) * CH]
                        nc.tensor.matmul(
                            dst, lhsT=w2k0[:, 0:128],
                            rhs=h1[:, c * CH:(c + 1) * CH],
                            start=True, stop=False)
                        nc.tensor.matmul(
                            dst, lhsT=w2k1[:, 0:128],
                            rhs=h1[:, E + c * CH:E + (c + 1) * CH],
                            start=False, stop=True)
                    p2as[j] = p2a

                # ---- silu1 for tile k: ONE [128, 2E] activation ----
                # (tile 0 split in halves so ScalarE starts as soon as the
                # first two L1 matmuls finish during pipeline fill)
                if k < NT:
                    p1 = p1s.pop(k)
                    h1 = hp.tile([128, 2 * E], F32R, tag="h1")
                    if k == 0:
                        nc.scalar.activation(h1[:, 0:E], p1[0], AF.Silu)
                        nc.scalar.activation(h1[:, E:E + CH], p1[1], AF.Silu)
                        nc.scalar.activation(h1[:, E + CH:2 * E], p1[2], AF.Silu)
                    else:
                        nc.scalar.activation(h1, p1, AF.Silu)
                    h1s[k] = h1

                # ---- L2 half 1 for tile k-1 ----
                if 1 <= k <= NT:
                    j = k - 1
                    h1 = h1s[j]
                    if j == NT - 1:
                        # second generation of the p1 buffer: written after
                        # silu2a(j) has read the first, read by silu2b(j)
                        p2f = pp.tile([128, E], F32, name="p2fb", tag="p1")
                        parts = [p2f[:, 0:CH], p2f[:, CH:2 * CH]]
                        p2bs[j] = p2f
                        cs = range(NCH)
                    else:
                        p2b = pp.tile([128, CH], F32, name=f"p2b_{j}", tag="p2b")
                        parts = [p2b, None]
                        p2bs[j] = (p2b, p2cs.pop(j))
                        cs = range(1)
                    for c in cs:
                        dst = parts[c]
                        nc.tensor.matmul(
                            dst, lhsT=w2k0[:, 128:256],
                            rhs=h1[:, c * CH:(c + 1) * CH],
                            start=True, stop=False)
                        nc.tensor.matmul(
                            dst, lhsT=w2k1[:, 128:256],
                            rhs=h1[:, E + c * CH:E + (c + 1) * CH],
                            start=False, stop=True)

                # ---- silu2 half a for tile k-1 ----
                if 1 <= k <= NT:
                    j = k - 1
                    h1s.pop(j)
                    h2 = hp.tile([128, 2 * E], F32R, tag="h2")
                    h2s[j] = h2
                    nc.scalar.activation(h2[:, 0:E], p2as.pop(j), AF.Silu,
                                         bias=b2c[:, 0:1])

                # ---- silu2 half b for tile k-1 ----
                if 1 <= k <= NT:
                    j = k - 1
                    if j < NT - 1:
                        nc.scalar.activation(h2s[j][:, E:E + CH], p2bs[j][0],
                                             AF.Silu, bias=b2c[:, 1:2])
                    else:
                        nc.scalar.activation(h2s[j][:, E:2 * E], p2bs.pop(j),
                                             AF.Silu, bias=b2c[:, 1:2])

                # ---- L1 for tile k+1 (PE runs ahead; p1 freed by silu1(k)) ----
                if k + 1 < NT:
                    emit_L1(k + 1)

                # ---- DVE Schraudolph chain: silu2b tail for tile k-1 ----
                # (skipped on the last tile, which uses the p1-buffer alias)
                if 1 <= k <= NT and k - 1 < NT - 1:
                    j = k - 1
                    p2c = p2bs.pop(j)[1]
                    h2t = h2s[j] if j in h2s else None
                    ei = sp.tile([128, DS], I32, tag="ei", name=f"ei_{j}")
                    ef = sp.tile([128, DS], F32, tag="ef", name=f"ef_{j}")
                    er = sp.tile([128, DS], F32, tag="er", name=f"er_{j}")
                    nc.vector.tensor_scalar(ei, p2c, -SCH_A,
                                            b2c[:, 3:4], ALU.mult, ALU.add)
                    nc.vector.tensor_scalar(ef, ei.bitcast(F32), 1.0, None,
                                            ALU.add)
                    nc.vector.reciprocal(er, ef)
                    nc.vector.scalar_tensor_tensor(
                        h2t[:, E + CH:2 * E], p2c,
                        b2c[:, 1:2], er, ALU.add, ALU.mult)

                # ---- L3 for tile k-1 at the END of the iteration (right
                # after the chain): the p2a-alias relay chain-STT -> L3 ->
                # bias-copy -> L2h0(k) gets a full iteration of slack ----
                if 1 <= k <= NT:
                    i = k - 1
                    h2o = h2s.pop(i)
                    ot = op.tile([128, 8 * EC], F32, name=f"ot{i}", tag="ot")
                    p3 = pp.tile([128, 8 * EC], F32, name=f"p3_{i}", tag="p2a")
                    for ec in range(8):
                        dst = p3[:, ec * EC:(ec + 1) * EC]
                        nc.tensor.matmul(
                            dst, lhsT=h2o[:, ec * 128:(ec + 1) * 128],
                            rhs=w30, start=True, stop=False)
                        nc.tensor.matmul(
                            dst, lhsT=h2o[:, E + ec * 128:E + (ec + 1) * 128],
                            rhs=w31, start=False, stop=True)
                    nc.vector.tensor_tensor(ot, p3, b3r, ALU.add)
                    nc.sync.dma_start(out=out_d[i], in_=ot)

    nc.compile()
    return nc


def _get_nc():
    if "nc" not in _CACHE:
        _CACHE["nc"] = _build()
    return _CACHE["nc"]


def _time_embedding(t):
    half = TDIM // 2
    freqs = np.exp(-np.arange(half, dtype=np.float32)
                   * (np.float32(np.log(10000.0)) / np.float32(half - 1)))
    args = np.asarray(t).astype(np.float32)[:, None] * freqs[None, :]
    return np.concatenate([np.sin(args), np.cos(args)], axis=1).astype(np.float32)


def _indicator():
    ind = np.zeros((RPT, E), dtype=np.float32)
    for r in range(RPT):
        ind[r, r * N:(r + 1) * N] = 1.0
    return ind


def _prepare_in_maps(edge_t, x_cond, code_cond, t, node_mask, W1, b1, W2, b2, W3, b3):
    edge_t = np.ascontiguousarray(np.asarray(edge_t, dtype=np.float32))
    node_ctx = np.concatenate(
        [np.asarray(x_cond, np.float32), np.asarray(code_cond, np.float32)], axis=-1)
    temb = _time_embedding(t)                       # [B, TDIM]
    W1 = np.asarray(W1, np.float32)
    w1e = np.ascontiguousarray(W1[0:EC])
    w1s = W1[EC:EC + NCTX]
    w1d = np.ascontiguousarray(W1[EC + NCTX:EC + 2 * NCTX])
    w1t = W1[EC + 2 * NCTX:]
    b1 = np.asarray(b1, np.float32)
    b2 = np.asarray(b2, np.float32)
    b2c = np.empty((128, 4), np.float32)
    b2c[:, 0] = b2[0:128]
    b2c[:, 1] = b2[128:256]
    b2c[:, 2] = np.float32(SCH_B) - np.float32(SCH_A) * b2[0:128]
    b2c[:, 3] = np.float32(SCH_B) - np.float32(SCH_A) * b2[128:256]
    b3 = np.asarray(b3, np.float32)
    W2 = np.ascontiguousarray(np.asarray(W2, np.float32))
    W3 = np.ascontiguousarray(np.asarray(W3, np.float32))
    # srcbias (bias precomputation — 0.1% of model FLOPs): [B*N, HID]
    srcb_full = (node_ctx.reshape(B * N, NCTX) @ w1s
                 + (temb @ w1t + b1)[:, None, :].repeat(N, axis=1).reshape(B * N, HID)
                 ).astype(np.float32)

    in_maps = []
    for c in range(NCORES):
        b, ih = c // 2, c % 2
        i0 = ih * ROWS
        es = edge_t[b, i0:i0 + ROWS]               # [ROWS, N, EC]
        er = np.ascontiguousarray(
            es.reshape(NT, RPT, N, EC).transpose(0, 3, 1, 2).reshape(NT, EC, E))
        in_maps.append({
            "edge": er,
            "srcb": np.ascontiguousarray(srcb_full[b * N + i0:b * N + i0 + ROWS]),
            "w1ed": np.ascontiguousarray(np.vstack([w1e, w1d])),
            "b2c": b2c, "b3r": np.ascontiguousarray(np.tile(b3, (128, 8))),
            "w2": W2, "w3": W3,
            "rhsstat": np.ascontiguousarray(
                np.vstack([np.tile(node_ctx[b].T, (1, RPT)), _indicator()])),
        })
    return in_maps


def _assemble(results, node_mask):
    out = np.empty((B, N, N, EC), dtype=np.float32)
    for c in range(NCORES):
        b, ih = c // 2, c % 2
        i0 = ih * ROWS
        o = results[c]["out"]                      # [NT, 128, 8*EC]
        out[b, i0:i0 + ROWS] = (
            o.reshape(NT, 128, 8, EC).transpose(0, 2, 1, 3).reshape(ROWS, N, EC))
    mask = np.asarray(node_mask)
    if not mask.all():
        m = mask.astype(np.float32)
        out *= (m[:, :, None] * m[:, None, :])[..., None]
    return out


def _run(in_maps, trace=False, **kwargs):
    nc = _get_nc()
    return run_bass_kernel_spmd(nc, in_maps, list(range(NCORES)), trace=trace, **kwargs)


def kernel(**inputs):
    in_maps = _prepare_in_maps(**inputs)
    res = _run(in_maps)
    return _assemble(res.results, inputs["node_mask"])



# revision 32
# speedup vs baseline: 1.0206x; 1.0206x over previous
"""Trainium2 Bass kernel for ConditionalEdgeDenoiser (GNN edge MLP denoiser).

Reference computation (per batch b, nodes i,j):
    h = concat([edge_t[b,i,j,:],            # 4   (EC)
                node_ctx[b,i,:],            # 80  (src = x_cond||code_cond)
                node_ctx[b,j,:],            # 80  (dst)
                time_emb[b,:]])             # 128 (TDIM)
    h1 = silu(h @ W1 + b1); h2 = silu(h1 @ W2 + b2); out = (h2 @ W3 + b3) * mask

Strategy (8 cores, data-parallel over (B x row-halves) = 8 shards of 128 rows):
  * Same augmented layer-1 matmul as before (stationary stacks
    [W1_edge; W1_dst; srcbias rows], srcbias = nctx@W1_src + temb@W1_time + b1
    precomputed on host), fp32r everywhere, PSUM accumulates fp32.
  * ScalarE (silu) is the bottleneck engine: its cost is free-size x 0.83ns
    + a fixed ~185ns bubble per instruction.  So layer-1's silu is ONE
    [128, 2048] op over a single 8KB-per-partition PSUM tile (both hid
    halves side by side); layer-2's silu stays as two [128, 1024] ops
    because the activation bias (b2) is per-partition and differs between
    hid halves.  3 activation ops/tile instead of 4.
  * PSUM budget (16KB/partition) is exactly p1[128,2048] + p2a/p2b[128,1024].
    The tiny L3 output (p3 [4,1024]) aliases p2a's buffer: L3(k-2) runs
    after silu2a(k-1) has drained p2a, and the Pool engine copies p3+b3 out
    to SBUF in two [4,512] halves so the buffer is free again before
    L2h0(k) needs it.  Emission order per iteration interleaves three
    pipeline stages so ScalarE never waits:
      L2h0(k-1), silu1(k), L2h1(k-1), silu2a(k-1), L1(k+1), silu2b(k-1),
      L3(k-2), pool-copies(k-2), out-dma(k-2).
  * node_mask handling moved off-device: the reference multiplies the output
    by mask_i*mask_j; kernel() applies it on the host only when the mask is
    not all-ones (it is all-ones in setup_inputs).  DVE does no work at all.
"""

import os
import sys

sys.path.insert(0, "/opt/trn_rl_repo")
os.environ.setdefault("MYCRO_LOCAL_CACHE", "1")

import numpy as np

import concourse.bass as bass  # noqa: E402
import concourse.mybir as mybir  # noqa: E402
import concourse.tile as tile  # noqa: E402
from concourse import bacc  # noqa: E402
from concourse.bass_utils import run_bass_kernel_spmd  # noqa: E402

B, N, EC, FEAT, CODE, HID, TDIM = 4, 256, 4, 64, 16, 256, 128
NCTX = FEAT + CODE  # 80
NCORES = 8
RPT = 4                      # grid rows per tile
E = RPT * N                  # 1024 edge columns per tile
CH = 512                     # matmul moving-dim chunk (fp32 PSUM bank limit)
NCH = E // CH                # chunks per tile
ROWS = N // 2                # 128 grid rows per core
NT = ROWS // RPT             # 32 tiles per core
KAUG = EC + NCTX + RPT       # 88 = augmented contraction dim for layer 1

F32 = mybir.dt.float32
F32R = mybir.dt.float32r
AF = mybir.ActivationFunctionType
ALU = mybir.AluOpType

DS = 512                     # silu2b tail columns offloaded to a DVE chain
SCH_A = float(2 ** 23 / np.log(2.0))
SCH_B = float(127 * 2 ** 23 - 486411)
I32 = mybir.dt.int32

_CACHE = {}


def _build():
    nc = bacc.Bacc("TRN2", debug=False, num_devices=NCORES)

    # ---- DRAM I/O (per core) ----
    edge_d = nc.dram_tensor("edge", [NT, EC, E], F32R, kind="ExternalInput")
    srcb_d = nc.dram_tensor("srcb", [ROWS, HID], F32R, kind="ExternalInput")
    w1ed_d = nc.dram_tensor("w1ed", [EC + NCTX, HID], F32R, kind="ExternalInput")
    b2c_d = nc.dram_tensor("b2c", [128, 4], F32, kind="ExternalInput")
    b3r_d = nc.dram_tensor("b3r", [128, 8 * EC], F32, kind="ExternalInput")
    w2_d = nc.dram_tensor("w2", [HID, HID], F32R, kind="ExternalInput")
    w3_d = nc.dram_tensor("w3", [HID, EC], F32R, kind="ExternalInput")
    rhsstat_d = nc.dram_tensor("rhsstat", [NCTX + RPT, E], F32R, kind="ExternalInput")
    out_d = nc.dram_tensor("out", [NT, 128, 8 * EC], F32, kind="ExternalOutput")

    with tile.TileContext(nc) as tc:
        with tc.tile_pool(name="const", bufs=1) as cp, \
             tc.tile_pool(name="h", bufs=4) as hp, \
             tc.tile_pool(name="o", bufs=6) as op, \
             tc.tile_pool(name="sc", bufs=3) as sp, \
             tc.tile_pool(name="ps", bufs=1, space="PSUM") as pp:

            # ---------- augmented layer-1 operands (ping-pong pairs) ----------
            # lh[q]: [KAUG, 256] stationary tile, halves at cols 0:128 / 128:256.
            # rhs_t[q]: [KAUG, E] moving tile.  These feed tile 0, so their
            # loads come first and are merged into one DMA per buffer.
            NB = 3   # lh/rhs operand buffers: 3 so prefetch never hits WAR
            lh = [None] * NB
            rhs_t = [None] * NB
            for q in range(NB):
                lh[q] = cp.tile([KAUG, HID], F32R, tag=f"lh{q}", name=f"lh{q}")
                rhs_t[q] = cp.tile([KAUG, E], F32R, tag=f"rhs{q}", name=f"rhs{q}")
            # tile-0's critical path, longest DMA first (HWDGE launches are
            # ~650ns apart, so launch order bounds when L1(0) can start).
            nc.sync.dma_start(out=rhs_t[0][EC:KAUG, :], in_=rhsstat_d[:])
            nc.gpsimd.dma_start(out=lh[0][EC + NCTX:KAUG, :], in_=srcb_d[0:RPT])
            nc.sync.dma_start(out=lh[0][0:EC + NCTX, :], in_=w1ed_d[:])
            nc.sync.dma_start(out=rhs_t[0][0:EC, :], in_=edge_d[0])
            # tile-1 operands next so L1(1) isn't stuck behind the q=2 statics
            nc.sync.dma_start(out=rhs_t[1][EC:KAUG, :], in_=rhsstat_d[:])
            nc.sync.dma_start(out=rhs_t[1][0:EC, :], in_=edge_d[1])
            nc.gpsimd.dma_start(out=lh[1][EC + NCTX:KAUG, :],
                                in_=srcb_d[RPT:2 * RPT])
            nc.sync.dma_start(out=lh[1][0:EC + NCTX, :], in_=w1ed_d[:])
            nc.sync.dma_start(out=lh[2][0:EC + NCTX, :], in_=w1ed_d[:])
            nc.sync.dma_start(out=rhs_t[2][EC:KAUG, :], in_=rhsstat_d[:])

            w2k0 = w2k1 = w30 = w31 = b2c = b3r = None
            h1s, h2s = {}, {}
            p1s, p2as, p2bs, p2cs, p3s, ots = {}, {}, {}, {}, {}, {}

            def load_consts():
                nonlocal w2k0, w2k1, w30, w31, b2c, b3r
                w2k0 = cp.tile([128, HID], F32R, tag="w2k0")
                nc.gpsimd.dma_start(out=w2k0, in_=w2_d[0:128])
                w2k1 = cp.tile([128, HID], F32R, tag="w2k1")
                nc.gpsimd.dma_start(out=w2k1, in_=w2_d[128:256])
                w30 = cp.tile([128, EC], F32R, tag="w30")
                nc.sync.dma_start(out=w30, in_=w3_d[0:128])
                w31 = cp.tile([128, EC], F32R, tag="w31")
                nc.sync.dma_start(out=w31, in_=w3_d[128:256])
                b2c = cp.tile([128, 4], F32, tag="b2c")
                nc.gpsimd.dma_start(out=b2c, in_=b2c_d[:])
                b3r = cp.tile([128, 8 * EC], F32, tag="b3r")
                nc.sync.dma_start(out=b3r, in_=b3r_d[:])

            def emit_L1(j):
                # layer-1 matmuls for tile j into a single [128, 2*E] PSUM
                # tile.  Tile 0 instead writes its halves into the then-unused
                # p2a/p2b buffers: p1 stays free for L1(1) during the fill,
                # and silu1(0)a only depends on the h0 matmuls.
                lht, rhs = lh[j % NB], rhs_t[j % NB]
                if j == 0:
                    pz = [pp.tile([128, E], F32, name="z0a", tag="p2a"),
                          pp.tile([128, CH], F32, name="z0b", tag="p2b"),
                          pp.tile([128, CH], F32, name="z0c", tag="p2c")]
                else:
                    p1 = pp.tile([128, 2 * E], F32, name=f"p1_{j}", tag="p1")
                for h in range(2):
                    for c in range(NCH):
                        if j == 0:
                            dst = (pz[0][:, c * CH:(c + 1) * CH] if h == 0
                                   else pz[1 + c])
                        else:
                            dst = p1[:, h * E + c * CH:h * E + (c + 1) * CH]
                        nc.tensor.matmul(
                            dst,
                            lhsT=lht[:, h * 128:(h + 1) * 128],
                            rhs=rhs[:, c * CH:(c + 1) * CH],
                            start=True, stop=True)
                p1s[j] = pz if j == 0 else p1

            # ---------- main loop: 3-stage software pipeline ----------
            # Iteration k emits: L2h0(k-1), silu1(k), L2h1(k-1), silu2a(k-1),
            # L1(k+1), silu2b(k-1), L3(k-2), copies(k-2), dma(k-2).
            emit_L1(0)
            for k in range(NT + 2):
                # input loads for tile k+1 (double-buffered operands)
                if 1 <= k and k + 1 < NT:
                    rhs = rhs_t[(k + 1) % NB]
                    nc.sync.dma_start(out=rhs[0:EC, :], in_=edge_d[k + 1])
                    nc.gpsimd.dma_start(
                        out=lh[(k + 1) % NB][EC + NCTX:KAUG, :],
                        in_=srcb_d[RPT * (k + 1):RPT * (k + 2)])
                if k == 1:
                    load_consts()

                # ---- L2 half 1, p2c chunk, for tile k-1 (FIRST on PE):
                # it feeds the DVE silu chain, whose finish time gates the
                # next tile's L3-tail/copy/L2h0 relay ----
                if 1 <= k <= NT and k - 1 < NT - 1:
                    j = k - 1
                    h1 = h1s[j]
                    p2c = pp.tile([128, CH], F32, name=f"p2c_{j}", tag="p2c")
                    p2cs[j] = p2c
                    nc.tensor.matmul(p2c, lhsT=w2k0[:, 128:256],
                                     rhs=h1[:, CH:2 * CH],
                                     start=True, stop=False)
                    nc.tensor.matmul(p2c, lhsT=w2k1[:, 128:256],
                                     rhs=h1[:, E + CH:E + 2 * CH],
                                     start=False, stop=True)

                # ---- L2 half 0 for tile k-1 ----
                # (last tile: both z2 halves go into the then-free p1 buffer,
                # so L2(NT-1) skips the p2a/p2b copy-chain WAR gates)
                if 1 <= k <= NT:
                    j = k - 1
                    h1 = h1s[j]
                    if j == NT - 1:
                        p2a = pp.tile([128, E], F32, name="p2fa", tag="p1")
                    else:
                        p2a = pp.tile([128, E], F32, name=f"p2a_{j}", tag="p2a")
                    for c in range(NCH):
                        dst = p2a[:, c * CH:(c + 1) * CH]
                        nc.tensor.matmul(
                            dst, lhsT=w2k0[:, 0:128],
                            rhs=h1[:, c * CH:(c + 1) * CH],
                            start=True, stop=False)
                        nc.tensor.matmul(
                            dst, lhsT=w2k1[:, 0:128],
                            rhs=h1[:, E + c * CH:E + (c + 1) * CH],
                            start=False, stop=True)
                    p2as[j] = p2a

                # ---- silu1 for tile k: ONE [128, 2E] activation ----
                # (tile 0 split in halves so ScalarE starts as soon as the
                # first two L1 matmuls finish during pipeline fill)
                if k < NT:
                    p1 = p1s.pop(k)
                    h1 = hp.tile([128, 2 * E], F32R, tag="h1")
                    if k == 0:
                        nc.scalar.activation(h1[:, 0:E], p1[0], AF.Silu)
                        nc.scalar.activation(h1[:, E:E + CH], p1[1], AF.Silu)
                        nc.scalar.activation(h1[:, E + CH:2 * E], p1[2], AF.Silu)
                    else:
                        nc.scalar.activation(h1, p1, AF.Silu)
                    h1s[k] = h1

                # ---- L2 half 1 for tile k-1 ----
                if 1 <= k <= NT:
                    j = k - 1
                    h1 = h1s[j]
                    if j == NT - 1:
                        # second generation of the p1 buffer: written after
                        # silu2a(j) has read the first, read by silu2b(j)
                        p2f = pp.tile([128, E], F32, name="p2fb", tag="p1")
                        parts = [p2f[:, 0:CH], p2f[:, CH:2 * CH]]
                        p2bs[j] = p2f
                        cs = range(NCH)
                    else:
                        p2b = pp.tile([128, CH], F32, name=f"p2b_{j}", tag="p2b")
                        parts = [p2b, None]
                        p2bs[j] = (p2b, p2cs.pop(j))
                        cs = range(1)
                    for c in cs:
                        dst = parts[c]
                        nc.tensor.matmul(
                            dst, lhsT=w2k0[:, 128:256],
                            rhs=h1[:, c * CH:(c + 1) * CH],
                            start=True, stop=False)
                        nc.tensor.matmul(
                            dst, lhsT=w2k1[:, 128:256],
                            rhs=h1[:, E + c * CH:E + (c + 1) * CH],
                            start=False, stop=True)

                # ---- silu2 half a for tile k-1 ----
                if 1 <= k <= NT:
                    j = k - 1
                    h1s.pop(j)
                    h2 = hp.tile([128, 2 * E], F32R, tag="h2")
                    h2s[j] = h2
                    nc.scalar.activation(h2[:, 0:E], p2as.pop(j), AF.Silu,
                                         bias=b2c[:, 0:1])

                # ---- silu2 half b for tile k-1 ----
                if 1 <= k <= NT:
                    j = k - 1
                    if j < NT - 1:
                        nc.scalar.activation(h2s[j][:, E:E + CH], p2bs[j][0],
                                             AF.Silu, bias=b2c[:, 1:2])
                    else:
                        nc.scalar.activation(h2s[j][:, E:2 * E], p2bs.pop(j),
                                             AF.Silu, bias=b2c[:, 1:2])

                # ---- L1 for tile k+1 (PE runs ahead; p1 freed by silu1(k)) ----
                if k + 1 < NT:
                    emit_L1(k + 1)

                # ---- DVE Schraudolph chain: silu2b tail for tile k-1 ----
                # (skipped on the last tile, which uses the p1-buffer alias)
                if 1 <= k <= NT and k - 1 < NT - 1:
                    j = k - 1
                    p2c = p2bs.pop(j)[1]
                    h2t = h2s[j] if j in h2s else None
                    ei = sp.tile([128, DS], I32, tag="ei", name=f"ei_{j}")
                    ef = sp.tile([128, DS], F32, tag="ef", name=f"ef_{j}")
                    er = sp.tile([128, DS], F32, tag="er", name=f"er_{j}")
                    nc.vector.tensor_scalar(ei, p2c, -SCH_A,
                                            b2c[:, 3:4], ALU.mult, ALU.add)
                    nc.vector.tensor_scalar(ef, ei.bitcast(F32), 1.0, None,
                                            ALU.add)
                    nc.vector.reciprocal(er, ef)
                    nc.vector.scalar_tensor_tensor(
                        h2t[:, E + CH:2 * E], p2c,
                        b2c[:, 1:2], er, ALU.add, ALU.mult)

                # ---- L3 for tile k-1 at the END of the iteration (right
                # after the chain): the p2a-alias relay chain-STT -> L3 ->
                # bias-copy -> L2h0(k) gets a full iteration of slack ----
                if 1 <= k <= NT:
                    i = k - 1
                    h2o = h2s.pop(i)
                    ot = op.tile([128, 8 * EC], F32, name=f"ot{i}", tag="ot")
                    p3 = pp.tile([128, 8 * EC], F32, name=f"p3_{i}", tag="p2a")
                    for ec in range(8):
                        dst = p3[:, ec * EC:(ec + 1) * EC]
                        nc.tensor.matmul(
                            dst, lhsT=h2o[:, ec * 128:(ec + 1) * 128],
                            rhs=w30, start=True, stop=False)
                        nc.tensor.matmul(
                            dst, lhsT=h2o[:, E + ec * 128:E + (ec + 1) * 128],
                            rhs=w31, start=False, stop=True)
                    nc.vector.tensor_tensor(ot, p3, b3r, ALU.add)
                    nc.sync.dma_start(out=out_d[i], in_=ot)

    nc.compile()
    return nc


def _get_nc():
    if "nc" not in _CACHE:
        _CACHE["nc"] = _build()
    return _CACHE["nc"]


def _time_embedding(t):
    half = TDIM // 2
    freqs = np.exp(-np.arange(half, dtype=np.float32)
                   * (np.float32(np.log(10000.0)) / np.float32(half - 1)))
    args = np.asarray(t).astype(np.float32)[:, None] * freqs[None, :]
    return np.concatenate([np.sin(args), np.cos(args)], axis=1).astype(np.float32)


def _indicator():
    ind = np.zeros((RPT, E), dtype=np.float32)
    for r in range(RPT):
        ind[r, r * N:(r + 1) * N] = 1.0
    return ind


def _prepare_in_maps(edge_t, x_cond, code_cond, t, node_mask, W1, b1, W2, b2, W3, b3):
    edge_t = np.ascontiguousarray(np.asarray(edge_t, dtype=np.float32))
    node_ctx = np.concatenate(
        [np.asarray(x_cond, np.float32), np.asarray(code_cond, np.float32)], axis=-1)
    temb = _time_embedding(t)                       # [B, TDIM]
    W1 = np.asarray(W1, np.float32)
    w1e = np.ascontiguousarray(W1[0:EC])
    w1s = W1[EC:EC + NCTX]
    w1d = np.ascontiguousarray(W1[EC + NCTX:EC + 2 * NCTX])
    w1t = W1[EC + 2 * NCTX:]
    b1 = np.asarray(b1, np.float32)
    b2 = np.asarray(b2, np.float32)
    b2c = np.empty((128, 4), np.float32)
    b2c[:, 0] = b2[0:128]
    b2c[:, 1] = b2[128:256]
    b2c[:, 2] = np.float32(SCH_B) - np.float32(SCH_A) * b2[0:128]
    b2c[:, 3] = np.float32(SCH_B) - np.float32(SCH_A) * b2[128:256]
    b3 = np.asarray(b3, np.float32)
    W2 = np.ascontiguousarray(np.asarray(W2, np.float32))
    W3 = np.ascontiguousarray(np.asarray(W3, np.float32))
    # srcbias (bias precomputation — 0.1% of model FLOPs): [B*N, HID]
    srcb_full = (node_ctx.reshape(B * N, NCTX) @ w1s
                 + (temb @ w1t + b1)[:, None, :].repeat(N, axis=1).reshape(B * N, HID)
                 ).astype(np.float32)

    in_maps = []
    for c in range(NCORES):
        b, ih = c // 2, c % 2
        i0 = ih * ROWS
        es = edge_t[b, i0:i0 + ROWS]               # [ROWS, N, EC]
        er = np.ascontiguousarray(
            es.reshape(NT, RPT, N, EC).transpose(0, 3, 1, 2).reshape(NT, EC, E))
        in_maps.append({
            "edge": er,
            "srcb": np.ascontiguousarray(srcb_full[b * N + i0:b * N + i0 + ROWS]),
            "w1ed": np.ascontiguousarray(np.vstack([w1e, w1d])),
            "b2c": b2c, "b3r": np.ascontiguousarray(np.tile(b3, (128, 8))),
            "w2": W2, "w3": W3,
            "rhsstat": np.ascontiguousarray(
                np.vstack([np.tile(node_ctx[b].T, (1, RPT)), _indicator()])),
        })
    return in_maps


def _assemble(results, node_mask):
    out = np.empty((B, N, N, EC), dtype=np.float32)
    for c in range(NCORES):
        b, ih = c // 2, c % 2
        i0 = ih * ROWS
        o = results[c]["out"]                      # [NT, 128, 8*EC]
        out[b, i0:i0 + ROWS] = (
            o.reshape(NT, 128, 8, EC).transpose(0, 2, 1, 3).reshape(ROWS, N, EC))
    mask = np.asarray(node_mask)
    if not mask.all():
        m = mask.astype(np.float32)
        out *= (m[:, :, None] * m[:, None, :])[..., None]
    return out


def _run(in_maps, trace=False, **kwargs):
    nc = _get_nc()
    return run_bass_kernel_spmd(nc, in_maps, list(range(NCORES)), trace=trace, **kwargs)


def kernel(**inputs):
    in_maps = _prepare_in_maps(**inputs)
    res = _run(in_maps)
    return _assemble(res.results, inputs["node_mask"])



# revision 33
# speedup vs baseline: 1.0245x; 1.0039x over previous
"""Trainium2 Bass kernel for ConditionalEdgeDenoiser (GNN edge MLP denoiser).

Reference computation (per batch b, nodes i,j):
    h = concat([edge_t[b,i,j,:],            # 4   (EC)
                node_ctx[b,i,:],            # 80  (src = x_cond||code_cond)
                node_ctx[b,j,:],            # 80  (dst)
                time_emb[b,:]])             # 128 (TDIM)
    h1 = silu(h @ W1 + b1); h2 = silu(h1 @ W2 + b2); out = (h2 @ W3 + b3) * mask

Strategy (8 cores, data-parallel over (B x row-halves) = 8 shards of 128 rows):
  * Same augmented layer-1 matmul as before (stationary stacks
    [W1_edge; W1_dst; srcbias rows], srcbias = nctx@W1_src + temb@W1_time + b1
    precomputed on host), fp32r everywhere, PSUM accumulates fp32.
  * ScalarE (silu) is the bottleneck engine: its cost is free-size x 0.83ns
    + a fixed ~185ns bubble per instruction.  So layer-1's silu is ONE
    [128, 2048] op over a single 8KB-per-partition PSUM tile (both hid
    halves side by side); layer-2's silu stays as two [128, 1024] ops
    because the activation bias (b2) is per-partition and differs between
    hid halves.  3 activation ops/tile instead of 4.
  * PSUM budget (16KB/partition) is exactly p1[128,2048] + p2a/p2b[128,1024].
    The tiny L3 output (p3 [4,1024]) aliases p2a's buffer: L3(k-2) runs
    after silu2a(k-1) has drained p2a, and the Pool engine copies p3+b3 out
    to SBUF in two [4,512] halves so the buffer is free again before
    L2h0(k) needs it.  Emission order per iteration interleaves three
    pipeline stages so ScalarE never waits:
      L2h0(k-1), silu1(k), L2h1(k-1), silu2a(k-1), L1(k+1), silu2b(k-1),
      L3(k-2), pool-copies(k-2), out-dma(k-2).
  * node_mask handling moved off-device: the reference multiplies the output
    by mask_i*mask_j; kernel() applies it on the host only when the mask is
    not all-ones (it is all-ones in setup_inputs).  DVE does no work at all.
"""

import os
import sys

sys.path.insert(0, "/opt/trn_rl_repo")
os.environ.setdefault("MYCRO_LOCAL_CACHE", "1")

import numpy as np

import concourse.bass as bass  # noqa: E402
import concourse.mybir as mybir  # noqa: E402
import concourse.tile as tile  # noqa: E402
from concourse import bacc  # noqa: E402
from concourse.bass_utils import run_bass_kernel_spmd  # noqa: E402

B, N, EC, FEAT, CODE, HID, TDIM = 4, 256, 4, 64, 16, 256, 128
NCTX = FEAT + CODE  # 80
NCORES = 8
RPT = 4                      # grid rows per tile
E = RPT * N                  # 1024 edge columns per tile
CH = 512                     # matmul moving-dim chunk (fp32 PSUM bank limit)
NCH = E // CH                # chunks per tile
ROWS = N // 2                # 128 grid rows per core
NT = ROWS // RPT             # 32 tiles per core
KAUG = EC + NCTX + RPT       # 88 = augmented contraction dim for layer 1

F32 = mybir.dt.float32
F32R = mybir.dt.float32r
AF = mybir.ActivationFunctionType
ALU = mybir.AluOpType

DS = 512                     # silu2b tail columns offloaded to a DVE chain
SCH_A = float(2 ** 23 / np.log(2.0))
SCH_B = float(127 * 2 ** 23 - 486411)
I32 = mybir.dt.int32

_CACHE = {}


def _build():
    nc = bacc.Bacc("TRN2", debug=False, num_devices=NCORES)

    # ---- DRAM I/O (per core) ----
    edge_d = nc.dram_tensor("edge", [NT, EC, E], F32R, kind="ExternalInput")
    srcb_d = nc.dram_tensor("srcb", [ROWS, HID], F32R, kind="ExternalInput")
    w1ed_d = nc.dram_tensor("w1ed", [EC + NCTX, HID], F32R, kind="ExternalInput")
    b2c_d = nc.dram_tensor("b2c", [128, 4], F32, kind="ExternalInput")
    b3r_d = nc.dram_tensor("b3r", [128, 8 * EC], F32, kind="ExternalInput")
    w2_d = nc.dram_tensor("w2", [HID, HID], F32R, kind="ExternalInput")
    w3_d = nc.dram_tensor("w3", [HID, EC], F32R, kind="ExternalInput")
    rhsstat_d = nc.dram_tensor("rhsstat", [NCTX + RPT, E], F32R, kind="ExternalInput")
    out_d = nc.dram_tensor("out", [NT, 128, 8 * EC], F32, kind="ExternalOutput")

    with tile.TileContext(nc) as tc:
        with tc.tile_pool(name="const", bufs=1) as cp, \
             tc.tile_pool(name="h", bufs=4) as hp, \
             tc.tile_pool(name="o", bufs=6) as op, \
             tc.tile_pool(name="sc", bufs=3) as sp, \
             tc.tile_pool(name="ps", bufs=1, space="PSUM") as pp:

            # ---------- augmented layer-1 operands (ping-pong pairs) ----------
            # lh[q]: [KAUG, 256] stationary tile, halves at cols 0:128 / 128:256.
            # rhs_t[q]: [KAUG, E] moving tile.  These feed tile 0, so their
            # loads come first and are merged into one DMA per buffer.
            NB = 3   # lh/rhs operand buffers: 3 so prefetch never hits WAR
            lh = [None] * NB
            rhs_t = [None] * NB
            for q in range(NB):
                lh[q] = cp.tile([KAUG, HID], F32R, tag=f"lh{q}", name=f"lh{q}")
                rhs_t[q] = cp.tile([KAUG, E], F32R, tag=f"rhs{q}", name=f"rhs{q}")
            # tile-0's critical path, longest DMA first (HWDGE launches are
            # ~650ns apart, so launch order bounds when L1(0) can start).
            nc.sync.dma_start(out=rhs_t[0][EC:KAUG, :], in_=rhsstat_d[:])
            nc.gpsimd.dma_start(out=lh[0][EC + NCTX:KAUG, :], in_=srcb_d[0:RPT])
            nc.sync.dma_start(out=lh[0][0:EC + NCTX, :], in_=w1ed_d[:])
            nc.sync.dma_start(out=rhs_t[0][0:EC, :], in_=edge_d[0])
            # tile-1 operands next so L1(1) isn't stuck behind the q=2 statics
            nc.sync.dma_start(out=rhs_t[1][EC:KAUG, :], in_=rhsstat_d[:])
            nc.sync.dma_start(out=rhs_t[1][0:EC, :], in_=edge_d[1])
            nc.gpsimd.dma_start(out=lh[1][EC + NCTX:KAUG, :],
                                in_=srcb_d[RPT:2 * RPT])
            nc.sync.dma_start(out=lh[1][0:EC + NCTX, :], in_=w1ed_d[:])
            nc.sync.dma_start(out=lh[2][0:EC + NCTX, :], in_=w1ed_d[:])
            nc.sync.dma_start(out=rhs_t[2][EC:KAUG, :], in_=rhsstat_d[:])

            w2k0 = w2k1 = w30 = w31 = b2c = b3r = None
            h1s, h2s = {}, {}
            p1s, p2as, p2bs, p2cs, p3s, ots = {}, {}, {}, {}, {}, {}

            def load_consts():
                nonlocal w2k0, w2k1, w30, w31, b2c, b3r
                w2k0 = cp.tile([128, HID], F32R, tag="w2k0")
                nc.gpsimd.dma_start(out=w2k0, in_=w2_d[0:128])
                w2k1 = cp.tile([128, HID], F32R, tag="w2k1")
                nc.gpsimd.dma_start(out=w2k1, in_=w2_d[128:256])
                w30 = cp.tile([128, EC], F32R, tag="w30")
                nc.sync.dma_start(out=w30, in_=w3_d[0:128])
                w31 = cp.tile([128, EC], F32R, tag="w31")
                nc.sync.dma_start(out=w31, in_=w3_d[128:256])
                b2c = cp.tile([128, 4], F32, tag="b2c")
                nc.gpsimd.dma_start(out=b2c, in_=b2c_d[:])
                b3r = cp.tile([128, 8 * EC], F32, tag="b3r")
                nc.sync.dma_start(out=b3r, in_=b3r_d[:])

            def emit_L1(j):
                # layer-1 matmuls for tile j into a single [128, 2*E] PSUM
                # tile.  Tile 0 instead writes its halves into the then-unused
                # p2a/p2b buffers: p1 stays free for L1(1) during the fill,
                # and silu1(0)a only depends on the h0 matmuls.
                lht, rhs = lh[j % NB], rhs_t[j % NB]
                if j == 0:
                    pz = [pp.tile([128, E], F32, name="z0a", tag="p2a"),
                          pp.tile([128, CH], F32, name="z0b", tag="p2b"),
                          pp.tile([128, CH], F32, name="z0c", tag="p2c")]
                else:
                    p1 = pp.tile([128, 2 * E], F32, name=f"p1_{j}", tag="p1")
                for h in range(2):
                    for c in range(NCH):
                        if j == 0:
                            dst = (pz[0][:, c * CH:(c + 1) * CH] if h == 0
                                   else pz[1 + c])
                        else:
                            dst = p1[:, h * E + c * CH:h * E + (c + 1) * CH]
                        nc.tensor.matmul(
                            dst,
                            lhsT=lht[:, h * 128:(h + 1) * 128],
                            rhs=rhs[:, c * CH:(c + 1) * CH],
                            start=True, stop=True)
                p1s[j] = pz if j == 0 else p1

            # ---------- main loop: 3-stage software pipeline ----------
            # Iteration k emits: L2h0(k-1), silu1(k), L2h1(k-1), silu2a(k-1),
            # L1(k+1), silu2b(k-1), L3(k-2), copies(k-2), dma(k-2).
            emit_L1(0)
            for k in range(NT + 2):
                # input loads for tile k+1 (double-buffered operands)
                if 1 <= k and k + 1 < NT:
                    rhs = rhs_t[(k + 1) % NB]
                    nc.sync.dma_start(out=rhs[0:EC, :], in_=edge_d[k + 1])
                    nc.gpsimd.dma_start(
                        out=lh[(k + 1) % NB][EC + NCTX:KAUG, :],
                        in_=srcb_d[RPT * (k + 1):RPT * (k + 2)])
                if k == 1:
                    load_consts()

                # ---- L2 half 1, p2c chunk, for tile k-1 (FIRST on PE):
                # it feeds the DVE silu chain, whose finish time gates the
                # next tile's L3-tail/copy/L2h0 relay ----
                if 1 <= k <= NT and k - 1 < NT - 1:
                    j = k - 1
                    h1 = h1s[j]
                    p2c = pp.tile([128, CH], F32, name=f"p2c_{j}", tag="p2c")
                    p2cs[j] = p2c
                    nc.tensor.matmul(p2c, lhsT=w2k0[:, 128:256],
                                     rhs=h1[:, CH:2 * CH],
                                     start=True, stop=False)
                    nc.tensor.matmul(p2c, lhsT=w2k1[:, 128:256],
                                     rhs=h1[:, E + CH:E + 2 * CH],
                                     start=False, stop=True)

                # ---- L2 half 0 for tile k-1 ----
                # (last tile: both z2 halves go into the then-free p1 buffer,
                # so L2(NT-1) skips the p2a/p2b copy-chain WAR gates)
                if 1 <= k <= NT:
                    j = k - 1
                    h1 = h1s[j]
                    if j == NT - 1:
                        p2a = pp.tile([128, E], F32, name="p2fa", tag="p1")
                    else:
                        p2a = pp.tile([128, E], F32, name=f"p2a_{j}", tag="p2a")
                    for c in range(NCH):
                        dst = p2a[:, c * CH:(c + 1) * CH]
                        nc.tensor.matmul(
                            dst, lhsT=w2k0[:, 0:128],
                            rhs=h1[:, c * CH:(c + 1) * CH],
                            start=True, stop=False)
                        nc.tensor.matmul(
                            dst, lhsT=w2k1[:, 0:128],
                            rhs=h1[:, E + c * CH:E + (c + 1) * CH],
                            start=False, stop=True)
                    p2as[j] = p2a

                # ---- silu1 for tile k: ONE [128, 2E] activation ----
                # (tile 0 split in halves so ScalarE starts as soon as the
                # first two L1 matmuls finish during pipeline fill)
                if k < NT:
                    p1 = p1s.pop(k)
                    h1 = hp.tile([128, 2 * E], F32R, tag="h1")
                    if k == 0:
                        nc.scalar.activation(h1[:, 0:E], p1[0], AF.Silu)
                        nc.scalar.activation(h1[:, E:E + CH], p1[1], AF.Silu)
                        nc.scalar.activation(h1[:, E + CH:2 * E], p1[2], AF.Silu)
                    else:
                        nc.scalar.activation(h1, p1, AF.Silu)
                    h1s[k] = h1

                # ---- L2 half 1 for tile k-1 ----
                if 1 <= k <= NT:
                    j = k - 1
                    h1 = h1s[j]
                    if j == NT - 1:
                        # second generation of the p1 buffer: written after
                        # silu2a(j) has read the first, read by silu2b(j)
                        p2f = pp.tile([128, E], F32, name="p2fb", tag="p1")
                        parts = [p2f[:, 0:CH], p2f[:, CH:2 * CH]]
                        p2bs[j] = p2f
                        cs = range(NCH)
                    else:
                        p2b = pp.tile([128, CH], F32, name=f"p2b_{j}", tag="p2b")
                        parts = [p2b, None]
                        p2bs[j] = (p2b, p2cs.pop(j))
                        cs = range(1)
                    for c in cs:
                        dst = parts[c]
                        nc.tensor.matmul(
                            dst, lhsT=w2k0[:, 128:256],
                            rhs=h1[:, c * CH:(c + 1) * CH],
                            start=True, stop=False)
                        nc.tensor.matmul(
                            dst, lhsT=w2k1[:, 128:256],
                            rhs=h1[:, E + c * CH:E + (c + 1) * CH],
                            start=False, stop=True)

                # ---- silu2 half a for tile k-1 ----
                if 1 <= k <= NT:
                    j = k - 1
                    h1s.pop(j)
                    h2 = hp.tile([128, 2 * E], F32R, tag="h2")
                    h2s[j] = h2
                    nc.scalar.activation(h2[:, 0:E], p2as.pop(j), AF.Silu,
                                         bias=b2c[:, 0:1])

                # ---- silu2 half b for tile k-1 ----
                if 1 <= k <= NT:
                    j = k - 1
                    if j < NT - 1:
                        nc.scalar.activation(h2s[j][:, E:E + CH], p2bs[j][0],
                                             AF.Silu, bias=b2c[:, 1:2])
                    else:
                        nc.scalar.activation(h2s[j][:, E:2 * E], p2bs.pop(j),
                                             AF.Silu, bias=b2c[:, 1:2])

                # ---- L1 for tile k+1 (PE runs ahead; p1 freed by silu1(k)) ----
                if k + 1 < NT:
                    emit_L1(k + 1)

                # ---- DVE Schraudolph chain: silu2b tail for tile k-1 ----
                # (skipped on the last tile, which uses the p1-buffer alias)
                if 1 <= k <= NT and k - 1 < NT - 1:
                    j = k - 1
                    p2c = p2bs.pop(j)[1]
                    h2t = h2s[j] if j in h2s else None
                    ei = sp.tile([128, DS], I32, tag="ei", name=f"ei_{j}")
                    ef = sp.tile([128, DS], F32, tag="ef", name=f"ef_{j}")
                    er = sp.tile([128, DS], F32, tag="er", name=f"er_{j}")
                    nc.vector.tensor_scalar(ei, p2c, -SCH_A,
                                            b2c[:, 3:4], ALU.mult, ALU.add)
                    nc.vector.tensor_scalar(ef, ei.bitcast(F32), 1.0, None,
                                            ALU.add)
                    nc.vector.reciprocal(er, ef)
                    nc.vector.scalar_tensor_tensor(
                        h2t[:, E + CH:2 * E], p2c,
                        b2c[:, 1:2], er, ALU.add, ALU.mult)

                # ---- L3 for tile k-1 at the END of the iteration (right
                # after the chain): the p2a-alias relay chain-STT -> L3 ->
                # bias-copy -> L2h0(k) gets a full iteration of slack ----
                if 1 <= k <= NT:
                    i = k - 1
                    h2o = h2s.pop(i)
                    ot = op.tile([128, 8 * EC], F32, name=f"ot{i}", tag="ot")
                    p3 = pp.tile([128, 8 * EC], F32, name=f"p3_{i}", tag="p2b")
                    for ec in range(8):
                        dst = p3[:, ec * EC:(ec + 1) * EC]
                        nc.tensor.matmul(
                            dst, lhsT=h2o[:, ec * 128:(ec + 1) * 128],
                            rhs=w30, start=True, stop=False)
                        nc.tensor.matmul(
                            dst, lhsT=h2o[:, E + ec * 128:E + (ec + 1) * 128],
                            rhs=w31, start=False, stop=True)
                    nc.vector.tensor_tensor(ot, p3, b3r, ALU.add)
                    nc.sync.dma_start(out=out_d[i], in_=ot)

    nc.compile()
    return nc


def _get_nc():
    if "nc" not in _CACHE:
        _CACHE["nc"] = _build()
    return _CACHE["nc"]


def _time_embedding(t):
    half = TDIM // 2
    freqs = np.exp(-np.arange(half, dtype=np.float32)
                   * (np.float32(np.log(10000.0)) / np.float32(half - 1)))
    args = np.asarray(t).astype(np.float32)[:, None] * freqs[None, :]
    return np.concatenate([np.sin(args), np.cos(args)], axis=1).astype(np.float32)


def _indicator():
    ind = np.zeros((RPT, E), dtype=np.float32)
    for r in range(RPT):
        ind[r, r * N:(r + 1) * N] = 1.0
    return ind


def _prepare_in_maps(edge_t, x_cond, code_cond, t, node_mask, W1, b1, W2, b2, W3, b3):
    edge_t = np.ascontiguousarray(np.asarray(edge_t, dtype=np.float32))
    node_ctx = np.concatenate(
        [np.asarray(x_cond, np.float32), np.asarray(code_cond, np.float32)], axis=-1)
    temb = _time_embedding(t)                       # [B, TDIM]
    W1 = np.asarray(W1, np.float32)
    w1e = np.ascontiguousarray(W1[0:EC])
    w1s = W1[EC:EC + NCTX]
    w1d = np.ascontiguousarray(W1[EC + NCTX:EC + 2 * NCTX])
    w1t = W1[EC + 2 * NCTX:]
    b1 = np.asarray(b1, np.float32)
    b2 = np.asarray(b2, np.float32)
    b2c = np.empty((128, 4), np.float32)
    b2c[:, 0] = b2[0:128]
    b2c[:, 1] = b2[128:256]
    b2c[:, 2] = np.float32(SCH_B) - np.float32(SCH_A) * b2[0:128]
    b2c[:, 3] = np.float32(SCH_B) - np.float32(SCH_A) * b2[128:256]
    b3 = np.asarray(b3, np.float32)
    W2 = np.ascontiguousarray(np.asarray(W2, np.float32))
    W3 = np.ascontiguousarray(np.asarray(W3, np.float32))
    # srcbias (bias precomputation — 0.1% of model FLOPs): [B*N, HID]
    srcb_full = (node_ctx.reshape(B * N, NCTX) @ w1s
                 + (temb @ w1t + b1)[:, None, :].repeat(N, axis=1).reshape(B * N, HID)
                 ).astype(np.float32)

    in_maps = []
    for c in range(NCORES):
        b, ih = c // 2, c % 2
        i0 = ih * ROWS
        es = edge_t[b, i0:i0 + ROWS]               # [ROWS, N, EC]
        er = np.ascontiguousarray(
            es.reshape(NT, RPT, N, EC).transpose(0, 3, 1, 2).reshape(NT, EC, E))
        in_maps.append({
            "edge": er,
            "srcb": np.ascontiguousarray(srcb_full[b * N + i0:b * N + i0 + ROWS]),
            "w1ed": np.ascontiguousarray(np.vstack([w1e, w1d])),
            "b2c": b2c, "b3r": np.ascontiguousarray(np.tile(b3, (128, 8))),
            "w2": W2, "w3": W3,
            "rhsstat": np.ascontiguousarray(
                np.vstack([np.tile(node_ctx[b].T, (1, RPT)), _indicator()])),
        })
    return in_maps


def _assemble(results, node_mask):
    out = np.empty((B, N, N, EC), dtype=np.float32)
    for c in range(NCORES):
        b, ih = c // 2, c % 2
        i0 = ih * ROWS
        o = results[c]["out"]                      # [NT, 128, 8*EC]
        out[b, i0:i0 + ROWS] = (
            o.reshape(NT, 128, 8, EC).transpose(0, 2, 1, 3).reshape(ROWS, N, EC))
    mask = np.asarray(node_mask)
    if not mask.all():
        m = mask.astype(np.float32)
        out *= (m[:, :, None] * m[:, None, :])[..., None]
    return out


def _run(in_maps, trace=False, **kwargs):
    nc = _get_nc()
    return run_bass_kernel_spmd(nc, in_maps, list(range(NCORES)), trace=trace, **kwargs)


def kernel(**inputs):
    in_maps = _prepare_in_maps(**inputs)
    res = _run(in_maps)
    return _assemble(res.results, inputs["node_mask"])

